# revision 9
# baseline (speedup 1.0000x reference)
"""Trainium2 Bass kernel for GatedGraphConv (Devign) GNN message passing.

Model (reference):
    h0 = pad(x, 256); 6 layers of: m = h @ w[l]; agg = scatter_add(m[src] -> dst);
    h = GRUCell(agg, h); then global mean pool per graph; 2-layer MLP classifier;
    sigmoid.

Distribution over 8 NeuronCores:
    - Nodes are partitioned contiguously: core c owns nodes [6250c, 6250(c+1)),
      padded to 6272 = 49*128 local slots.
    - h lives SBUF-resident, feature-major (hT [256, nodes]) split into
      per-(feature-chunk, node-block) tiles.
    - Per layer: m = h @ w computed locally (row-major), AllGather of m so every
      core holds the full message table, then each core gathers its in-edge
      messages by indirect DMA and scatter-adds them with a one-hot matmul
      (edges pre-sorted by destination on the host; S^T blocks built on host).
    - GRU gates: PSUM-fused matmuls (w_ih branch and w_hh branch accumulate
      into one PSUM group), sigmoid/tanh on the scalar engine, elementwise on
      the vector engine.
    - Mean-pool via one-hot matmul with host-prescaled 1/count entries,
      AllReduce of the [256,256] graph sums, classifier on-device.

The graded entry point is kernel(**inputs) -> np.ndarray [256, 1] float32.
"""

import numpy as np

import concourse.bacc as bacc
import concourse.bass as bass
import concourse.mybir as mybir
import concourse.tile as tile
from concourse.masks import make_identity

# Problem constants (hardcoded per the harness contract).
N_CORES = 8
N_NODES = 50000
N_EDGES = 300000
IN_DIM = 128
C = 256            # out_channels
G3 = 3 * C         # gru gate width
L = 6              # layers
N_GRAPHS = 256
NLOC = N_NODES // N_CORES      # 6250 real nodes per core
TPC = (NLOC + 127) // 128      # 49 dst tiles per core
NLP = TPC * 128                # 6272 padded local nodes
NFULL = N_CORES * NLP          # 50176 rows in the gathered message table
P = 128

# GRU node blocks: 12x512 + 1x128 = 6272.
BLOCKS = [(i * 512, 512) for i in range(12)] + [(12 * 512, 128)]

F32 = mybir.dt.float32
I32 = mybir.dt.int32


# --------------------------------------------------------------------------
# Host-side preprocessing
# --------------------------------------------------------------------------

def preprocess(x, edge_index, batch, weight, w_ih, w_hh, b_ih, b_hh, W1, b1, W2, b2):
    x = np.asarray(x, np.float32)
    edge_index = np.asarray(edge_index, np.int64)
    batch = np.asarray(batch, np.int64)
    weight = np.asarray(weight, np.float32)
    w_ih = np.asarray(w_ih, np.float32)
    w_hh = np.asarray(w_hh, np.float32)
    b_ih = np.asarray(b_ih, np.float32)
    b_hh = np.asarray(b_hh, np.float32)
    W1 = np.asarray(W1, np.float32)
    b1 = np.asarray(b1, np.float32)
    W2 = np.asarray(W2, np.float32)
    b2 = np.asarray(b2, np.float32)

    src = edge_index[0]
    dst = edge_index[1]

    # Per-(core, tile) edge grouping by destination.
    core_of = dst // NLOC
    loc = dst - core_of * NLOC
    t_of = loc // P
    r_of = loc - t_of * P
    # m_full row index of each source node.
    src_row = (src // NLOC) * NLP + (src % NLOC)

    # Bucket edges.
    order = np.lexsort((r_of, t_of, core_of))
    core_s = core_of[order]
    t_s = t_of[order]
    r_s = r_of[order].astype(np.int32)
    srow_s = src_row[order].astype(np.int32)

    # Segment boundaries per (core, tile).
    key = core_s * TPC + t_s
    counts = np.bincount(key, minlength=N_CORES * TPC)
    max_cnt = counts.max()
    nchunk = int((max_cnt + P - 1) // P)
    epad = nchunk * P
    starts = np.zeros(N_CORES * TPC, np.int64)
    np.cumsum(counts[:-1], out=starts[1:])

    gidx = np.zeros((N_CORES, TPC, P, nchunk), np.int32)
    s_t = np.zeros((N_CORES, TPC, nchunk, P, P), np.float32)
    for c in range(N_CORES):
        for t in range(TPC):
            k0 = starts[c * TPC + t]
            n = counts[c * TPC + t]
            rows = srow_s[k0 : k0 + n]
            rloc = r_s[k0 : k0 + n]
            j = np.arange(n)
            kk = j // P
            ee = j - kk * P
            gidx[c, t, ee, kk] = rows
            s_t[c, t, kk, ee, rloc] = 1.0

    # Pooling matrix, prescaled by 1/count.
    gcount = np.bincount(batch, minlength=N_GRAPHS).astype(np.float32)
    inv = 1.0 / np.maximum(gcount, 1.0)
    bpool = np.zeros((N_CORES, TPC, P, N_GRAPHS), np.float32)
    nodes = np.arange(N_NODES)
    bc = nodes // NLOC
    bl = nodes - bc * NLOC
    bt = bl // P
    bp = bl - bt * P
    bpool[bc, bt, bp, batch] = inv[batch]

    # h0^T per core: [256, NLP].
    h0T = np.zeros((N_CORES, C, NLP), np.float32)
    for c in range(N_CORES):
        h0T[c, :IN_DIM, :NLOC] = x[c * NLOC : (c + 1) * NLOC].T

    # Weights (replicated).
    wz = weight.reshape(L, 2, P, C).copy()          # w[l][c-chunk] as [128,256]
    wihT = np.ascontiguousarray(w_ih.T).reshape(2, P, G3)
    whhT = np.ascontiguousarray(w_hh.T).reshape(2, P, G3)
    # bias columns: 0,1=r (b_ih+b_hh); 2,3=z (b_ih+b_hh); 4,5=i_n (b_ih);
    # 6,7=h_n (b_hh). Shape [128, 8].
    bsum = b_ih + b_hh
    bias = np.stack(
        [bsum[0:128], bsum[128:256], bsum[256:384], bsum[384:512],
         b_ih[512:640], b_ih[640:768], b_hh[512:640], b_hh[640:768]], axis=1
    ).astype(np.float32)
    w1T = np.ascontiguousarray(W1.T).reshape(2, P, 128)
    w2T = np.ascontiguousarray(W2.T)                 # [128, 1]
    b1c = b1.reshape(P, 1)
    b2c = b2.reshape(1, 1).astype(np.float32)

    in_maps = []
    for c in range(N_CORES):
        in_maps.append({
            "h0T": h0T[c],
            "gidx": gidx[c],
            "s_t": s_t[c].reshape(TPC, nchunk * P, P),
            "bpool": bpool[c],
            "wz": wz,
            "wihT": wihT,
            "whhT": whhT,
            "bias": bias,
            "w1T": w1T,
            "b1": b1c,
            "w2T": w2T,
            "b2": b2c,
        })
    return in_maps, nchunk


# --------------------------------------------------------------------------
# Device kernel
# --------------------------------------------------------------------------

def build_kernel(nchunk: int, debug: bool = False):
    nc = bacc.Bacc(None, num_devices=N_CORES)

    h0T_in = nc.dram_tensor("h0T", [C, NLP], F32, kind="ExternalInput")
    gidx_in = nc.dram_tensor("gidx", [TPC, P, nchunk], I32, kind="ExternalInput")
    s_t_in = nc.dram_tensor("s_t", [TPC, nchunk * P, P], F32, kind="ExternalInput")
    bpool_in = nc.dram_tensor("bpool", [TPC, P, N_GRAPHS], F32, kind="ExternalInput")
    wz_in = nc.dram_tensor("wz", [L, 2, P, C], F32, kind="ExternalInput")
    wihT_in = nc.dram_tensor("wihT", [2, P, G3], F32, kind="ExternalInput")
    whhT_in = nc.dram_tensor("whhT", [2, P, G3], F32, kind="ExternalInput")
    bias_in = nc.dram_tensor("bias", [P, 8], F32, kind="ExternalInput")
    w1T_in = nc.dram_tensor("w1T", [2, P, 128], F32, kind="ExternalInput")
    b1_in = nc.dram_tensor("b1", [P, 1], F32, kind="ExternalInput")
    w2T_in = nc.dram_tensor("w2T", [P, 1], F32, kind="ExternalInput")
    b2_in = nc.dram_tensor("b2", [1, 1], F32, kind="ExternalInput")
    out = nc.dram_tensor("out", [1, N_GRAPHS], F32, kind="ExternalOutput")
    if debug:
        dbg_m = nc.dram_tensor("dbg_m", [NLP, C], F32, kind="ExternalOutput")
        dbg_agg = nc.dram_tensor("dbg_agg", [C, NLP], F32, kind="ExternalOutput")
        dbg_h = nc.dram_tensor("dbg_h", [C, NLP], F32, kind="ExternalOutput")

    rg = [list(range(N_CORES))]

    with tile.TileContext(nc) as tc:
        with (
            tc.tile_pool(name="persist", bufs=1) as pp,
            tc.tile_pool(name="msb", bufs=2) as msb_pool,
            tc.tile_pool(name="mg", bufs=2) as mg_pool,
            tc.tile_pool(name="ssb", bufs=3) as ssb_pool,
            tc.tile_pool(name="gsb", bufs=10) as gsb_pool,
            tc.tile_pool(name="ps", bufs=6, space="PSUM") as ps_pool,
            tc.tile_pool(name="pps", bufs=1, space="PSUM") as pps_pool,
            tc.tile_pool(name="dram", bufs=1, space="DRAM") as dr,
        ):
            m_loc = dr.tile([NLP, C], F32, name="m_loc")
            m_fulls = [
                dr.tile([NFULL, C], F32, addr_space="Shared", name=f"m_full_{l}")
                for l in range(L)
            ]
            sums_loc = dr.tile([C, N_GRAPHS], F32, name="sums_loc")
            sums_full = dr.tile([C, N_GRAPHS], F32, addr_space="Shared",
                                name="sums_full")

            # ---------------- persistent SBUF state ----------------
            h_t = [[None] * len(BLOCKS) for _ in range(2)]
            agg_t = [[None] * len(BLOCKS) for _ in range(2)]
            for cch in range(2):
                for b, (off, nb) in enumerate(BLOCKS):
                    h_t[cch][b] = pp.tile([P, nb], F32, name=f"h_{cch}_{b}")
                    nc.sync.dma_start(
                        out=h_t[cch][b][:],
                        in_=h0T_in[cch * P : (cch + 1) * P, off : off + nb],
                    )
                    agg_t[cch][b] = pp.tile([P, nb], F32, name=f"agg_{cch}_{b}")

            w_sb = pp.tile([P, L * 2 * C], F32, name="w_sb")
            for l in range(L):
                for cch in range(2):
                    nc.sync.dma_start(
                        out=w_sb[:, (l * 2 + cch) * C : (l * 2 + cch + 1) * C],
                        in_=wz_in[l, cch],
                    )
            wih_sb = pp.tile([P, 2 * G3], F32, name="wih_sb")
            whh_sb = pp.tile([P, 2 * G3], F32, name="whh_sb")
            for cch in range(2):
                nc.sync.dma_start(out=wih_sb[:, cch * G3 : (cch + 1) * G3],
                                  in_=wihT_in[cch])
                nc.sync.dma_start(out=whh_sb[:, cch * G3 : (cch + 1) * G3],
                                  in_=whhT_in[cch])
            bias_sb = pp.tile([P, 8], F32, name="bias_sb")
            nc.sync.dma_start(out=bias_sb[:], in_=bias_in[:])
            w1_sb = pp.tile([P, 2 * 128], F32, name="w1_sb")
            for cch in range(2):
                nc.sync.dma_start(out=w1_sb[:, cch * 128 : (cch + 1) * 128],
                                  in_=w1T_in[cch])
            b1_sb = pp.tile([P, 1], F32, name="b1_sb")
            nc.sync.dma_start(out=b1_sb[:], in_=b1_in[:])
            w2_sb = pp.tile([P, 1], F32, name="w2_sb")
            nc.sync.dma_start(out=w2_sb[:], in_=w2T_in[:])
            b2_sb = pp.tile([1, 1], F32, name="b2_sb")
            nc.sync.dma_start(out=b2_sb[:], in_=b2_in[:])

            gidx_sb = pp.tile([P, TPC * nchunk], I32, name="gidx_sb")
            nc.sync.dma_start(
                out=gidx_sb[:].rearrange("p (t k) -> p t k", k=nchunk),
                in_=gidx_in.rearrange("t p k -> p t k"),
            )
            ident_sb = pp.tile([P, P], F32, name="ident_sb")
            make_identity(nc, ident_sb[:])

            def wih(cch, gc):
                return wih_sb[:, cch * G3 + gc * P : cch * G3 + (gc + 1) * P]

            def whh(cch, gc):
                return whh_sb[:, cch * G3 + gc * P : cch * G3 + (gc + 1) * P]

            def h_slice(cch, t):
                b = t // 4
                o = (t % 4) * P
                return h_t[cch][b][:, o : o + P]

            # ---------------- layers ----------------
            for l in range(L):
                # Phase A: m = h @ w[l], row-major, stored to m_loc per block.
                for b, (off, nb) in enumerate(BLOCKS):
                    nt = nb // P
                    m_blk = msb_pool.tile([P, nt * C], F32, name="m_blk")
                    for q in range(nt):
                        t = b * 4 + q
                        psm = ps_pool.tile([P, C], F32, name="psm", tag="ps")
                        for cch in range(2):
                            nc.tensor.matmul(
                                psm[:],
                                lhsT=h_slice(cch, t),
                                rhs=w_sb[:, (l * 2 + cch) * C : (l * 2 + cch + 1) * C],
                                start=(cch == 0),
                                stop=(cch == 1),
                            )
                        nc.scalar.copy(m_blk[:, q * C : (q + 1) * C], psm[:])
                    nc.sync.dma_start(
                        out=m_loc[off : off + nb, :].rearrange(
                            "(q p) f -> p q f", p=P
                        ),
                        in_=m_blk[:].rearrange("p (q f) -> p q f", f=C),
                    )

                # Phase B: AllGather m -> m_full.
                m_full = m_fulls[l]
                nc.gpsimd.collective_compute(
                    "AllGather",
                    mybir.AluOpType.bypass,
                    replica_groups=rg,
                    ins=[m_loc[:]],
                    outs=[m_full[:]],
                )

                # Phase C: gather + one-hot matmul aggregation per dst tile.
                for t in range(TPC):
                    m_g = mg_pool.tile([P, nchunk * C], F32, name="m_g")
                    # HW indirect DMA consumes ONE offset per partition (the
                    # sim's multi-offset semantics do not match hardware), so
                    # gather each 128-edge chunk with its own instruction.
                    for k in range(nchunk):
                        nc.gpsimd.indirect_dma_start(
                            out=m_g[:, k * C : (k + 1) * C],
                            out_offset=None,
                            in_=m_full[:],
                            in_offset=bass.IndirectOffsetOnAxis(
                                ap=gidx_sb[:, t * nchunk + k : t * nchunk + k + 1],
                                axis=0,
                            ),
                        )
                    s_sb = ssb_pool.tile([P, nchunk * P], F32, name="s_sb")
                    nc.sync.dma_start(
                        out=s_sb[:].rearrange("e (k r) -> e k r", r=P),
                        in_=s_t_in[t].rearrange("(k e) r -> e k r", e=P),
                    )
                    bq = t // 4
                    oq = (t % 4) * P
                    for fh in range(2):
                        psa = ps_pool.tile([P, P], F32, name="psa", tag="ps")
                        for k in range(nchunk):
                            nc.tensor.matmul(
                                psa[:],
                                lhsT=m_g[:, k * C + fh * P : k * C + fh * P + P],
                                rhs=s_sb[:, k * P : (k + 1) * P],
                                start=(k == 0),
                                stop=(k == nchunk - 1),
                            )
                        nc.vector.tensor_copy(
                            agg_t[fh][bq][:, oq : oq + P], psa[:]
                        )

                if debug and l == 0:
                    nc.sync.dma_start(out=dbg_m[:], in_=m_loc[:])
                    for cch in range(2):
                        for b2, (off2, nb2) in enumerate(BLOCKS):
                            nc.sync.dma_start(
                                out=dbg_agg[cch * P : (cch + 1) * P, off2 : off2 + nb2],
                                in_=agg_t[cch][b2][:],
                            )

                # Phase D: GRU per node block. The h' writes are deferred
                # until both feature halves' gate matmuls have been traced:
                # Tile dependencies follow trace order, so writing h_t[0]
                # before gh=1's matmuls would feed them the updated h.
                for b, (off, nb) in enumerate(BLOCKS):
                    z_keep = [None, None]
                    n_keep = [None, None]
                    for gh in range(2):
                        # r gate: fused ih+hh accumulation.
                        psr = ps_pool.tile([P, nb], F32, name="psr", tag="ps")
                        nc.tensor.matmul(psr[:], lhsT=wih(0, gh), rhs=agg_t[0][b][:],
                                         start=True, stop=False)
                        nc.tensor.matmul(psr[:], lhsT=wih(1, gh), rhs=agg_t[1][b][:],
                                         start=False, stop=False)
                        nc.tensor.matmul(psr[:], lhsT=whh(0, gh), rhs=h_t[0][b][:],
                                         start=False, stop=False)
                        nc.tensor.matmul(psr[:], lhsT=whh(1, gh), rhs=h_t[1][b][:],
                                         start=False, stop=True)
                        r_sb = gsb_pool.tile([P, nb], F32, name="r_sb", tag="gate")
                        nc.scalar.activation(
                            r_sb[:], psr[:], mybir.ActivationFunctionType.Sigmoid,
                            bias=bias_sb[:, gh : gh + 1],
                        )
                        # z gate.
                        psz = ps_pool.tile([P, nb], F32, name="psz", tag="ps")
                        nc.tensor.matmul(psz[:], lhsT=wih(0, 2 + gh),
                                         rhs=agg_t[0][b][:], start=True, stop=False)
                        nc.tensor.matmul(psz[:], lhsT=wih(1, 2 + gh),
                                         rhs=agg_t[1][b][:], start=False, stop=False)
                        nc.tensor.matmul(psz[:], lhsT=whh(0, 2 + gh),
                                         rhs=h_t[0][b][:], start=False, stop=False)
                        nc.tensor.matmul(psz[:], lhsT=whh(1, 2 + gh),
                                         rhs=h_t[1][b][:], start=False, stop=True)
                        z_sb = gsb_pool.tile([P, nb], F32, name="z_sb", tag="gate")
                        nc.scalar.activation(
                            z_sb[:], psz[:], mybir.ActivationFunctionType.Sigmoid,
                            bias=bias_sb[:, 2 + gh : 3 + gh],
                        )
                        # i_n partial.
                        psi = ps_pool.tile([P, nb], F32, name="psi", tag="ps")
                        nc.tensor.matmul(psi[:], lhsT=wih(0, 4 + gh),
                                         rhs=agg_t[0][b][:], start=True, stop=False)
                        nc.tensor.matmul(psi[:], lhsT=wih(1, 4 + gh),
                                         rhs=agg_t[1][b][:], start=False, stop=True)
                        # h_n partial.
                        psh = ps_pool.tile([P, nb], F32, name="psh", tag="ps")
                        nc.tensor.matmul(psh[:], lhsT=whh(0, 4 + gh),
                                         rhs=h_t[0][b][:], start=True, stop=False)
                        nc.tensor.matmul(psh[:], lhsT=whh(1, 4 + gh),
                                         rhs=h_t[1][b][:], start=False, stop=True)
                        hn_sb = gsb_pool.tile([P, nb], F32, name="hn_sb", tag="gate")
                        nc.scalar.activation(
                            hn_sb[:], psh[:], mybir.ActivationFunctionType.Identity,
                            bias=bias_sb[:, 6 + gh : 7 + gh],
                        )
                        rn_sb = gsb_pool.tile([P, nb], F32, name="rn_sb", tag="gate")
                        nc.vector.tensor_mul(rn_sb[:], r_sb[:], hn_sb[:])
                        tn_sb = gsb_pool.tile([P, nb], F32, name="tn_sb", tag="gate")
                        nc.vector.tensor_add(tn_sb[:], psi[:], rn_sb[:])
                        n_sb = gsb_pool.tile([P, nb], F32, name="n_sb", tag="gate")
                        nc.scalar.activation(
                            n_sb[:], tn_sb[:], mybir.ActivationFunctionType.Tanh,
                            bias=bias_sb[:, 4 + gh : 5 + gh],
                        )
                        z_keep[gh] = z_sb
                        n_keep[gh] = n_sb
                    # h' = n + z*(h - n), traced after all reads of old h.
                    for gh in range(2):
                        d_sb = gsb_pool.tile([P, nb], F32, name="d_sb", tag="gate")
                        nc.vector.tensor_sub(d_sb[:], h_t[gh][b][:], n_keep[gh][:])
                        zd_sb = gsb_pool.tile([P, nb], F32, name="zd_sb", tag="gate")
                        nc.vector.tensor_mul(zd_sb[:], z_keep[gh][:], d_sb[:])
                        nc.vector.tensor_add(h_t[gh][b][:], n_keep[gh][:], zd_sb[:])

                if debug and l == 0:
                    for cch in range(2):
                        for b2, (off2, nb2) in enumerate(BLOCKS):
                            nc.sync.dma_start(
                                out=dbg_h[cch * P : (cch + 1) * P, off2 : off2 + nb2],
                                in_=h_t[cch][b2][:],
                            )

            # ---------------- pooling ----------------
            pp0 = pps_pool.tile([P, N_GRAPHS], F32, name="pp0")
            pp1 = pps_pool.tile([P, N_GRAPHS], F32, name="pp1")
            ppx = [pp0, pp1]
            for t in range(TPC):
                h_rm = msb_pool.tile([P, C], F32, name="h_rm")
                for cch in range(2):
                    ptr = ps_pool.tile([P, P], F32, name="ptr", tag="ps")
                    nc.tensor.transpose(ptr[:], h_slice(cch, t), ident_sb[:])
                    nc.vector.tensor_copy(h_rm[:, cch * P : (cch + 1) * P], ptr[:])
                b_sb = ssb_pool.tile([P, N_GRAPHS], F32, name="b_sb")
                nc.sync.dma_start(out=b_sb[:], in_=bpool_in[t])
                for fh in range(2):
                    nc.tensor.matmul(
                        ppx[fh][:],
                        lhsT=h_rm[:, fh * P : (fh + 1) * P],
                        rhs=b_sb[:],
                        start=(t == 0),
                        stop=(t == TPC - 1),
                    )
            sums_sb = pp.tile([P, 2 * N_GRAPHS], F32, name="sums_sb")
            nc.scalar.copy(sums_sb[:, 0:N_GRAPHS], pp0[:])
            nc.scalar.copy(sums_sb[:, N_GRAPHS : 2 * N_GRAPHS], pp1[:])
            nc.sync.dma_start(out=sums_loc[0:P, :], in_=sums_sb[:, 0:N_GRAPHS])
            nc.sync.dma_start(out=sums_loc[P : 2 * P, :],
                              in_=sums_sb[:, N_GRAPHS : 2 * N_GRAPHS])

            nc.gpsimd.collective_compute(
                "AllReduce",
                mybir.AluOpType.add,
                replica_groups=rg,
                ins=[sums_loc[:]],
                outs=[sums_full[:]],
            )

            # ---------------- classifier ----------------
            mt_sb = pp.tile([P, 2 * N_GRAPHS], F32, name="mt_sb")
            nc.sync.dma_start(out=mt_sb[:, 0:N_GRAPHS], in_=sums_full[0:P, :])
            nc.sync.dma_start(out=mt_sb[:, N_GRAPHS : 2 * N_GRAPHS],
                              in_=sums_full[P : 2 * P, :])
            ps1 = ps_pool.tile([P, N_GRAPHS], F32, name="ps1", tag="ps")
            for cch in range(2):
                nc.tensor.matmul(
                    ps1[:],
                    lhsT=w1_sb[:, cch * 128 : (cch + 1) * 128],
                    rhs=mt_sb[:, cch * N_GRAPHS : (cch + 1) * N_GRAPHS],
                    start=(cch == 0),
                    stop=(cch == 1),
                )
            h1_sb = pp.tile([P, N_GRAPHS], F32, name="h1_sb")
            nc.scalar.activation(
                h1_sb[:], ps1[:], mybir.ActivationFunctionType.Relu,
                bias=b1_sb[:, 0:1],
            )
            ps2 = ps_pool.tile([1, N_GRAPHS], F32, name="ps2", tag="ps")
            nc.tensor.matmul(ps2[:], lhsT=w2_sb[:, 0:1], rhs=h1_sb[:],
                             start=True, stop=True)
            out_sb = pp.tile([1, N_GRAPHS], F32, name="out_sb")
            nc.scalar.activation(
                out_sb[:], ps2[:], mybir.ActivationFunctionType.Sigmoid,
                bias=b2_sb[:, 0:1],
            )
            nc.sync.dma_start(out=out[:], in_=out_sb[:])

    nc.finalize()
    return nc


# --------------------------------------------------------------------------
# PJRT SPMD runner
# --------------------------------------------------------------------------

class SpmdRunner:
    def __init__(self, nc, n_cores):
        import jax
        from jax.experimental.shard_map import shard_map
        from jax.sharding import Mesh, PartitionSpec
        from concourse.bass2jax import (
            _bass_exec_p,
            install_neuronx_cc_hook,
            partition_id_tensor,
        )

        install_neuronx_cc_hook()
        self.jax = jax
        self.nc = nc
        self.n_cores = n_cores

        partition_name = nc.partition_id_tensor.name if nc.partition_id_tensor else None
        in_names, out_names, out_avals, zero_outs = [], [], [], []
        for alloc in nc.m.functions[0].allocations:
            if not isinstance(alloc, mybir.MemoryLocationSet):
                continue
            if not alloc.memorylocations:
                continue
            name = alloc.memorylocations[0].name
            if alloc.kind == "ExternalInput":
                if name != partition_name:
                    in_names.append(name)
            elif alloc.kind == "ExternalOutput":
                shape = tuple(alloc.tensor_shape)
                dtype = mybir.dt.np(alloc.dtype)
                out_names.append(name)
                out_avals.append(jax.core.ShapedArray(shape, dtype))
                zero_outs.append(np.zeros(shape, dtype))
        self.in_names = in_names
        self.out_names = out_names
        self.out_avals = out_avals
        self.zero_outs = zero_outs
        n_params = len(in_names)
        n_outs = len(out_names)
        all_in_names = list(in_names) + list(out_names)
        if partition_name is not None:
            all_in_names.append(partition_name)

        def _body(*args):
            operands = list(args)
            if partition_name is not None:
                operands.append(partition_id_tensor())
            outs = _bass_exec_p.bind(
                *operands,
                out_avals=tuple(out_avals),
                in_names=tuple(all_in_names),
                out_names=tuple(out_names),
                lowering_input_output_aliases=(),
                sim_require_finite=True,
                sim_require_nnan=True,
                nc=nc,
            )
            return tuple(outs)

        devices = jax.devices()[:n_cores]
        assert len(devices) == n_cores, (
            f"need {n_cores} neuron cores, found {len(jax.devices())}"
        )
        mesh = Mesh(np.asarray(devices), ("core",))
        in_specs = (PartitionSpec("core"),) * (n_params + n_outs)
        out_specs = (PartitionSpec("core"),) * n_outs
        self.fn = jax.jit(
            shard_map(_body, mesh=mesh, in_specs=in_specs, out_specs=out_specs,
                      check_rep=False),
            keep_unused=True,
        )

    def prepare(self, in_maps):
        n = self.n_cores
        concat_in = [
            np.concatenate([np.asarray(in_maps[c][name]) for c in range(n)], axis=0)
            for name in self.in_names
        ]
        concat_zero = [
            np.zeros((n * z.shape[0], *z.shape[1:]), z.dtype) for z in self.zero_outs
        ]
        return [self.jax.device_put(a) for a in concat_in + concat_zero]

    def run(self, dev_args):
        outs = self.fn(*dev_args)
        self.jax.block_until_ready(outs)
        return outs

    def results(self, outs):
        n = self.n_cores
        return [
            {
                name: np.asarray(outs[i]).reshape(n, *self.out_avals[i].shape)[c]
                for i, name in enumerate(self.out_names)
            }
            for c in range(n)
        ]


_RUNNER_CACHE = {}


def get_runner(nchunk: int) -> SpmdRunner:
    if nchunk not in _RUNNER_CACHE:
        _RUNNER_CACHE[nchunk] = SpmdRunner(build_kernel(nchunk), N_CORES)
    return _RUNNER_CACHE[nchunk]


def kernel(**inputs) -> np.ndarray:
    in_maps, nchunk = preprocess(**inputs)
    runner = get_runner(nchunk)
    dev_args = runner.prepare(in_maps)
    outs = runner.run(dev_args)
    res = runner.results(outs)
    probs = np.asarray(res[0]["out"], np.float32).reshape(1, N_GRAPHS)
    return probs.T.copy()
